# revision 1
# baseline (speedup 1.0000x reference)
"""Trainium2 Bass kernel for nn_EntropyLM (wavelet-coeff mixer + chunked MHA + output proj).

Strategy: data-parallel over the 16 independent (batch x chunk) blocks, 2 per
NeuronCore.  All matmuls run in bf16 on the PE with fp32 PSUM accumulation;
layernorm / softmax statistics are computed in fp32.

Layout convention per chunk (CHUNK=1024 tokens, H=1024 features):
  * Linear layers contract over features, so the activation operand of each
    matmul must be feature-major ("T" tensors: [feat_part, token_free]).
  * LN / softmax reductions run along the free axis, so those stages use
    token-major tensors ([token_part, feat_free]).
  * Attention scores are computed directly transposed (ST = K @ Q^T, i.e.
    [k_part, q_free]); exp(ST) is then exactly the lhsT operand that the
    PV matmul needs, which avoids any on-chip transpose of the score matrix.
    The softmax denominator is computed with a ones-vector matmul (partition
    reduction on the PE) and applied per-partition after PV.
  * Orientation changes of bf16 activations go through the DMA xbar
    transpose engine (dma_start_transpose), never through the PE.
"""

import numpy as np
import ml_dtypes

B, S, H, G, W = 4, 4096, 1024, 256, 8
CHUNK = 1024
NUM_HEADS = 4
HD = H // NUM_HEADS          # 256 per-head dim
HM = H // 2                  # 512 mixer hidden
N_CHUNKS = B * (S // CHUNK)  # 16 independent chunks
N_CORES = 8
CPC = N_CHUNKS // N_CORES    # 2 chunks per core
NT = CHUNK // 128            # 8 token tiles
KH = H // 128                # 8 feature tiles (H)
KM = HM // 128               # 4 feature tiles (HM)
EPS = 1e-5
BF16 = ml_dtypes.bfloat16

_COMPILED = None


def _build(debug=False):
    import concourse.bass as bass  # noqa: F401
    import concourse.tile as tile
    from concourse import bacc, mybir

    bf = mybir.dt.bfloat16
    fp16 = mybir.dt.float16
    f32 = mybir.dt.float32
    Alu = mybir.AluOpType
    Act = mybir.ActivationFunctionType

    nc = bacc.Bacc("TRN2", target_bir_lowering=False, debug=False,
                   enable_asserts=True, num_devices=N_CORES)

    # ---- DRAM tensors (per-core views; same NEFF on all 8 cores) ----
    xt = nc.dram_tensor("xt", [CPC, H, CHUNK], bf, kind="ExternalInput")
    kernT = nc.dram_tensor("kernt", [H, W], bf, kind="ExternalInput")
    w1a = nc.dram_tensor("w1a", [W + 1, HM], bf, kind="ExternalInput")
    gln = nc.dram_tensor("gln", [128, KM], f32, kind="ExternalInput")
    bln = nc.dram_tensor("bln", [128, KM], f32, kind="ExternalInput")
    w2 = nc.dram_tensor("w2", [HM, H], bf, kind="ExternalInput")
    b2c = nc.dram_tensor("b2c", [128, KH], f32, kind="ExternalInput")
    wq = nc.dram_tensor("wq", [H, H], bf, kind="ExternalInput")
    wk = nc.dram_tensor("wk", [H, H], bf, kind="ExternalInput")
    wv = nc.dram_tensor("wv", [H, H], bf, kind="ExternalInput")
    wo = nc.dram_tensor("wo", [H, H], bf, kind="ExternalInput")
    gw = nc.dram_tensor("gw", [H, G], bf, kind="ExternalInput")
    bw = nc.dram_tensor("bw", [128, G], f32, kind="ExternalInput")
    y = nc.dram_tensor("y", [CPC, CHUNK, G], f32, kind="ExternalOutput")
    dbg = {}
    if debug:
        for nm, shp, dt in [
            ("dcoef", [W + 1, CHUNK], bf),
            ("dhidT", [128, KM, CHUNK], bf), ("dmixT", [128, KH, CHUNK], bf),
            ("dmixN", [128, NT, H], bf), ("dqT", [128, KH, CHUNK], bf),
            ("dkT", [128, KH, CHUNK], bf), ("dvN", [128, NT, H], fp16),
            ("det", [128, KH, CHUNK], fp16), ("docat", [128, NT, H], bf),
            ("dres", [128, NT, H], bf), ("dz", [128, NT, H], bf),
            ("dzT", [128, KH, CHUNK], bf), ("dsq", [128, NT], f32),
        ]:
            dbg[nm] = nc.dram_tensor(nm, shp, dt, kind="ExternalOutput")

    with tile.TileContext(nc) as tc:
        with (
            tc.tile_pool(name="wp", bufs=1) as wp,
            tc.tile_pool(name="ws", bufs=1) as ws,
            tc.tile_pool(name="sm", bufs=2) as sm,
            tc.tile_pool(name="ps", bufs=3, space="PSUM") as ps,
            tc.tile_pool(name="ps2", bufs=2, space="PSUM") as ps2,
        ):
            # ---------- persistent weights ----------
            kt_sb = wp.tile([128, KH, W], bf, tag="ktw")
            nc.sync.dma_start(kt_sb[:], kernT.ap().rearrange("(i p) w -> p i w", p=128))
            w1a_sb = wp.tile([W + 1, HM], bf, tag="w1a")
            nc.sync.dma_start(w1a_sb[:], w1a.ap())
            gln_sb = wp.tile([128, KM], f32, tag="gln")
            nc.sync.dma_start(gln_sb[:], gln.ap())
            bln_sb = wp.tile([128, KM], f32, tag="bln")
            nc.sync.dma_start(bln_sb[:], bln.ap())
            b2_sb = wp.tile([128, KH], f32, tag="b2")
            nc.sync.dma_start(b2_sb[:], b2c.ap())
            gw_sb = wp.tile([128, KH, G], bf, tag="gw")
            nc.sync.dma_start(gw_sb[:], gw.ap().rearrange("(i p) g -> p i g", p=128))
            bw_sb = wp.tile([128, G], f32, tag="bw")
            nc.sync.dma_start(bw_sb[:], bw.ap())
            ones_sb = wp.tile([128, 1], fp16, tag="ones")
            nc.vector.memset(ones_sb[:], 1.0)
            eps_sb = wp.tile([128, 1], f32, tag="eps")
            nc.vector.memset(eps_sb[:], EPS)

            def stream_w(src):
                dst = ws.tile([128, KH, H], bf, tag="wstream", bufs=2, name="wst")
                nc.sync.dma_start(dst[:], src.ap().rearrange("(i p) m -> p i m", p=128))
                return dst

            # ---------- stage 1 (both chunks up front): wavelet coeffs ----------
            # Running chunk 1's input load + tiny coeff matmuls during chunk 0's
            # mixer window removes the chunk-boundary DMA stall.
            coefs = []
            for c in range(CPC):
                xts = ws.tile([128, KH, CHUNK], bf, tag="xts_et", bufs=2)
                for ii in range(2):
                    nc.sync.dma_start(
                        xts[:, ii * 4:(ii + 1) * 4, :],
                        xt.ap()[c, ii * 512:(ii + 1) * 512, :].rearrange(
                            "(i p) t -> p i t", p=128))
                coef = ws.tile([W + 1, CHUNK], bf, tag="coef", bufs=2)
                # row W is the constant 1.0 bias row for the folded mix_b1
                nc.gpsimd.memset(coef[:, :], 1.0)
                for n in range(2):
                    cps = ps.tile([128, 512], f32, tag="mm")
                    for i in range(KH):
                        nc.tensor.matmul(cps[:W, :], kt_sb[:, i, :],
                                         xts[:, i, n * 512:(n + 1) * 512],
                                         start=(i == 0), stop=(i == KH - 1))
                    nc.scalar.copy(coef[:W, n * 512:(n + 1) * 512], cps[:W, :])
                coefs.append(coef)

            for c in range(CPC):
                coef = coefs[c]
                w2s = ws.tile([128, KM, H], bf, tag="wstream", bufs=2, name="w2s")
                nc.sync.dma_start(w2s[:], w2.ap().rearrange("(i p) m -> p i m", p=128))
                wq_sb = stream_w(wq)
                wk_sb = stream_w(wk)
                if debug and c == 0:
                    nc.sync.dma_start(dbg["dcoef"].ap(), coef[:])
                # ---------- stage 2: mixer hidden + LN + gelu -> hidT ----------
                # z1 = (pre-m)*inv in token-major (stats per-partition), then
                # transpose; gamma/beta + gelu applied feature-major where
                # they are per-partition -> one fused TS + in-place gelu.
                hidT = ws.tile([128, KM, CHUNK], bf, tag="hidT")
                for t in range(NT):
                    hps = ps.tile([128, 512], f32, tag="mm")
                    nc.tensor.matmul(hps[:], coef[:, t * 128:(t + 1) * 128],
                                     w1a_sb[:], start=True, stop=True)
                    st6 = sm.tile([128, 6], f32, tag="st6")
                    nc.vector.bn_stats(st6[:], hps[:])
                    mv = sm.tile([128, 2], f32, tag="mv")
                    nc.vector.bn_aggr(mv[:], st6[:])
                    sq = sm.tile([128, 1], f32, tag="sq")
                    nc.scalar.activation(sq[:], mv[:, 1:2], Act.Sqrt, bias=eps_sb[:])
                    iv = sm.tile([128, 1], f32, tag="iv")
                    nc.vector.reciprocal(iv[:], sq[:])
                    tmp = sm.tile([128, HM], bf, tag="mtmp")
                    nc.vector.tensor_scalar(tmp[:], hps[:],
                                            mv[:, 0:1], iv[:],
                                            op0=Alu.subtract, op1=Alu.mult)
                    nc.sync.dma_start_transpose(hidT[:, :, t * 128:(t + 1) * 128],
                                                tmp[:])
                for nh in range(2):
                    for ki in range(KM):
                        sl = hidT[:, ki, nh * 512:(nh + 1) * 512]
                        nc.vector.tensor_scalar(sl, sl,
                                                gln_sb[:, ki:ki + 1], bln_sb[:, ki:ki + 1],
                                                op0=Alu.mult, op1=Alu.add)
                        nc.scalar.activation(sl, sl, Act.Gelu)

                if debug and c == 0:
                    nc.sync.dma_start(dbg["dhidT"].ap(), hidT[:])
                # ---------- stage 3: mixedT (+b2) and mixed_nat ----------
                mixT = ws.tile([128, KH, CHUNK], bf, tag="mixT_z", bufs=2)
                for n in range(2):
                    for m in range(KH):
                        mps = ps.tile([128, 512], f32, tag="mm")
                        for ki in range(KM):
                            nc.tensor.matmul(mps[:], w2s[:, ki, m * 128:(m + 1) * 128],
                                             hidT[:, ki, n * 512:(n + 1) * 512],
                                             start=(ki == 0), stop=(ki == KM - 1))
                        nc.vector.tensor_scalar(mixT[:, m, n * 512:(n + 1) * 512], mps[:],
                                                b2_sb[:, m:m + 1], None, op0=Alu.add)
                mixN = ws.tile([128, NT, H], bf, tag="mixN")
                for m in range(KH):
                    nc.sync.dma_start_transpose(mixN[:, :, m * 128:(m + 1) * 128],
                                                mixT[:, m, :])

                if debug and c == 0:
                    nc.sync.dma_start(dbg["dmixT"].ap(), mixT[:])
                    nc.sync.dma_start(dbg["dmixN"].ap(), mixN[:])
                # ---------- stage 4: qT, kT, v ----------
                qT = ws.tile([128, KH, CHUNK], bf, tag="qT_otc")
                kT = ws.tile([128, KH, CHUNK], bf, tag="kT_zT")
                for (dst, wsb, on_act) in ((qT, wq_sb, True), (kT, wk_sb, False)):
                    for n in range(2):
                        for m in range(KH):
                            qps = ps.tile([128, 512], f32, tag="mm")
                            for ki in range(KH):
                                nc.tensor.matmul(qps[:], wsb[:, ki, m * 128:(m + 1) * 128],
                                                 mixT[:, ki, n * 512:(n + 1) * 512],
                                                 start=(ki == 0), stop=(ki == KH - 1))
                            if on_act:
                                nc.scalar.copy(dst[:, m, n * 512:(n + 1) * 512], qps[:])
                            else:
                                nc.vector.tensor_copy(dst[:, m, n * 512:(n + 1) * 512], qps[:])
                wv_sb = stream_w(wv)
                vN = ws.tile([128, NT, H], fp16, tag="hp_v")
                for t in range(NT):
                    for n in range(2):
                        vps = ps.tile([128, 512], f32, tag="mm")
                        for ki in range(KH):
                            nc.tensor.matmul(vps[:], mixT[:, ki, t * 128:(t + 1) * 128],
                                             wv_sb[:, ki, n * 512:(n + 1) * 512],
                                             start=(ki == 0), stop=(ki == KH - 1))
                        nc.scalar.copy(vN[:, t, n * 512:(n + 1) * 512], vps[:])

                if debug and c == 0:
                    nc.sync.dma_start(dbg["dqT"].ap(), qT[:])
                    nc.sync.dma_start(dbg["dkT"].ap(), kT[:])
                    nc.sync.dma_start(dbg["dvN"].ap(), vN[:])
                wo_sb = stream_w(wo)
                # ---------- stage 5: attention ----------
                ocat = ws.tile([128, NT, H], bf, tag="hidT_oc_res")
                if debug and c == 0:
                    dsq_sb = sm.tile([128, NT], f32, tag="dsq")
                for h in range(NUM_HEADS):
                    et = ws.tile([128, KH, CHUNK], fp16, tag="xts_et", bufs=2)
                    for kt in range(NT):
                        stp = ps2.tile([128, CHUNK], f32, tag="st")
                        for qn in range(2):
                            for dk in range(2):
                                nc.tensor.matmul(
                                    stp[:, qn * 512:(qn + 1) * 512],
                                    kT[:, 2 * h + dk, kt * 128:(kt + 1) * 128],
                                    qT[:, 2 * h + dk, qn * 512:(qn + 1) * 512],
                                    start=(dk == 0), stop=(dk == 1))
                        # exp(score/sqrt(hd)); values are O(1e-1) so no max-sub needed
                        nc.scalar.activation(et[:, kt, :], stp[:], Act.Exp,
                                             scale=float(HD ** -0.5))
                    for qt in range(NT):
                        ovp = ps.tile([128, 512], f32, tag="mm")
                        for kt in range(NT):
                            # O_unnorm[q, d] accumulation; the extra N=1 matmul
                            # with a ones column gives s[q] = sum_k exp in the
                            # same [q_part, 1] orientation the normalization
                            # needs (same lhsT -> weight load is reused).
                            nc.tensor.matmul(ovp[:, :HD], et[:, kt, qt * 128:(qt + 1) * 128],
                                             vN[:, kt, h * HD:(h + 1) * HD],
                                             start=(kt == 0), stop=(kt == NT - 1))
                            # start=False even at kt==0: start=True clears the
                            # whole PSUM bank and would wipe the V-matmul's
                            # kt==0 contribution.  The bank-clear from the
                            # V-matmul above leaves this column's has_written
                            # bits 0, so kt==0 overwrites (not accumulates).
                            nc.tensor.matmul(ovp[:, HD:HD + 1],
                                             et[:, kt, qt * 128:(qt + 1) * 128],
                                             ones_sb[:],
                                             start=False, stop=(kt == NT - 1),
                                             skip_group_check=True)
                        rq = sm.tile([128, 1], f32, tag="rq")
                        if debug and c == 0 and h == NUM_HEADS - 1:
                            nc.vector.tensor_copy(dsq_sb[:, qt:qt + 1], ovp[:, HD:HD + 1])
                        nc.vector.reciprocal(rq[:], ovp[:, HD:HD + 1])
                        nc.vector.tensor_scalar(ocat[:, qt, h * HD:(h + 1) * HD],
                                                ovp[:, :HD], rq[:], None,
                                                op0=Alu.mult)
                otc = ws.tile([128, KH, CHUNK], bf, tag="qT_otc")
                for qt in range(NT):
                    nc.sync.dma_start_transpose(otc[:, :, qt * 128:(qt + 1) * 128],
                                                ocat[:, qt, :])

                if debug and c == 0:
                    nc.sync.dma_start(dbg["det"].ap(), et[:])
                    nc.sync.dma_start(dbg["docat"].ap(), ocat[:])
                    nc.sync.dma_start(dbg["dsq"].ap(), dsq_sb[:])
                # ---------- stage 6: wo proj + residual + out LN ----------
                res = ws.tile([128, NT, H], bf, tag="hidT_oc_res")
                z = ws.tile([128, NT, H], bf, tag="mixT_z", bufs=2)
                zT = ws.tile([128, KH, CHUNK], bf, tag="kT_zT")
                for t in range(NT):
                    for n in range(2):
                        ops_ = ps.tile([128, 512], f32, tag="mm")
                        for fi in range(KH):
                            nc.tensor.matmul(ops_[:], otc[:, fi, t * 128:(t + 1) * 128],
                                             wo_sb[:, fi, n * 512:(n + 1) * 512],
                                             start=(fi == 0), stop=(fi == KH - 1))
                        nc.vector.tensor_add(res[:, t, n * 512:(n + 1) * 512], ops_[:],
                                             mixN[:, t, n * 512:(n + 1) * 512])
                    st6 = sm.tile([128, 2, 6], f32, tag="st6b")
                    for half in range(2):
                        nc.vector.bn_stats(st6[:, half, :],
                                           res[:, t, half * 512:(half + 1) * 512])
                    mv = sm.tile([128, 2], f32, tag="mv")
                    nc.vector.bn_aggr(mv[:], st6[:])
                    sq = sm.tile([128, 1], f32, tag="sq")
                    nc.scalar.activation(sq[:], mv[:, 1:2], Act.Sqrt, bias=eps_sb[:])
                    iv = sm.tile([128, 1], f32, tag="iv")
                    nc.vector.reciprocal(iv[:], sq[:])
                    nc.vector.tensor_scalar(z[:, t, :], res[:, t, :],
                                            mv[:, 0:1], iv[:],
                                            op0=Alu.subtract, op1=Alu.mult)
                    nc.sync.dma_start_transpose(zT[:, :, t * 128:(t + 1) * 128],
                                                z[:, t, :])

                if debug and c == 0:
                    nc.sync.dma_start(dbg["dres"].ap(), res[:])
                    nc.sync.dma_start(dbg["dz"].ap(), z[:])
                    nc.sync.dma_start(dbg["dzT"].ap(), zT[:])
                # ---------- stage 7: output projection ----------
                ych = ws.tile([128, NT, G], f32, tag="ych", bufs=1)
                for t in range(NT):
                    yps = ps.tile([128, 512], f32, tag="mm")
                    for fi in range(KH):
                        nc.tensor.matmul(yps[:, :G], zT[:, fi, t * 128:(t + 1) * 128],
                                         gw_sb[:, fi, :],
                                         start=(fi == 0), stop=(fi == KH - 1))
                    nc.vector.tensor_add(ych[:, t, :], yps[:, :G], bw_sb[:])
                for hh in range(2):
                    nc.sync.dma_start(
                        y.ap()[c, hh * 512:(hh + 1) * 512, :].rearrange(
                            "(t p) g -> p t g", p=128),
                        ych[:, hh * 4:(hh + 1) * 4, :])

    nc.compile()
    return nc


def _get_compiled():
    global _COMPILED
    if _COMPILED is None:
        _COMPILED = _build()
    return _COMPILED


def _prep_inputs(inputs):
    f32 = np.float32

    def a(name):
        return np.asarray(inputs[name], dtype=f32)

    x = a("x")
    mw = a("mother_wavelets")
    scales = a("scales")
    norm = np.sqrt(np.sum(mw ** 2, axis=2, keepdims=True))
    kern = (mw / np.maximum(norm, 1e-12)) * (1.0 / (1.0 + np.exp(-scales)))
    kern = kern[0, :, :, 0]                      # (W, H)
    kernT = np.ascontiguousarray(kern.T).astype(BF16)

    w1a = np.concatenate([a("mix_w1"), a("mix_b1")[None, :]], axis=0).astype(BF16)
    gln = np.ascontiguousarray(a("mix_ln_g").reshape(KM, 128).T).astype(f32)
    bln = np.ascontiguousarray(a("mix_ln_b").reshape(KM, 128).T).astype(f32)
    w2 = a("mix_w2").astype(BF16)
    b2c = np.ascontiguousarray(a("mix_b2").reshape(KH, 128).T).astype(f32)
    gw = (a("out_ln_g")[:, None] * a("out_w")).astype(BF16)
    bw_vec = a("out_ln_b") @ a("out_w") + a("out_b")
    bw = np.tile(bw_vec[None, :], (128, 1)).astype(f32)

    shared = {
        "kernt": kernT, "w1a": w1a, "gln": gln, "bln": bln, "w2": w2,
        "b2c": b2c, "wq": a("wq").astype(BF16), "wk": a("wk").astype(BF16),
        "wv": a("wv").astype(BF16), "wo": a("wo").astype(BF16),
        "gw": gw, "bw": bw,
    }

    xc = x.reshape(N_CHUNKS, CHUNK, H)
    xt_all = np.ascontiguousarray(xc.transpose(0, 2, 1)).astype(BF16)  # (16, H, CHUNK)
    in_maps = []
    for core in range(N_CORES):
        m = dict(shared)
        m["xt"] = np.ascontiguousarray(xt_all[core * CPC:(core + 1) * CPC])
        in_maps.append(m)
    return in_maps


def kernel(**inputs) -> np.ndarray:
    from concourse.bass_utils import run_bass_kernel_spmd

    nc = _get_compiled()
    in_maps = _prep_inputs(inputs)
    res = run_bass_kernel_spmd(nc, in_maps, core_ids=list(range(N_CORES)))
    out = np.concatenate([r["y"] for r in res.results], axis=0)  # (16, CHUNK, G)
    return out.reshape(B, S, G).astype(np.float32)



# revision 10
# speedup vs baseline: 1.4664x; 1.4664x over previous
"""Trainium2 Bass kernel for nn_EntropyLM (wavelet-coeff mixer + chunked MHA + output proj).

Data-parallel over the 16 independent (batch x chunk) blocks, 2 per core.

Precision plan (validated in numpy, predicted rel-err ~8.4e-3 vs 2e-2 gate):
  * Mixer path (coef, w1, w2) and output projection in fp16 on the PE.
  * Attention path (q/k/v, scores, PV, wo) in fp8-e4m3 with DoubleRow
    matmuls: two stacked 128-deep K-subtiles at 0.5 cycles/row.
  * fp8 range handling: weights pre-scaled by 64, activations rescaled on
    PSUM evacuation; the residual branch carries an 8192x scale that LN2
    absorbs (eps scaled to match); the softmax-denominator ones vector is
    1/32 so PV normalization applies the ocat fp8 range scale for free.
  * Residual add (wo_out + mixed) is done on the PE by accumulating an
    identity matmul of mixN into the wo PSUM group; LN2 stats and z read
    straight from PSUM, so `res` never materializes in SBUF.

Scheduling: the two chunks are software-pipelined by emission order (engine
queues are FIFO): chunk1's mixer stages (PE/DVE-heavy) are emitted inside
chunk0's attention window (Act/exp-bound), and chunk1's attention overlaps
chunk0's output stages.  PSUM: 512-wide `ps` pool for GEMMs interleaved
with attention; 1024-wide `ps2` pool for q/k/v GEMMs, score tiles, and the
wo+LN2 groups (freed per-token-tile so rotation never clobbers live data).
"""

import numpy as np
import ml_dtypes

B, S, H, G, W = 4, 4096, 1024, 256, 8
CHUNK = 1024
NUM_HEADS = 4
HD = H // NUM_HEADS          # 256 per-head dim
HM = H // 2                  # 512 mixer hidden
N_CHUNKS = B * (S // CHUNK)  # 16 independent chunks
N_CORES = 8
CPC = N_CHUNKS // N_CORES    # 2 chunks per core
NT = CHUNK // 128            # 8 token tiles
KH = H // 128                # 8 feature tiles (H)
KM = HM // 128               # 4 feature tiles (HM)
KP = KH // 2                 # 4 double-row K pairs over H
EPS = 1e-5
SC_RES = 8192.0              # residual-branch scale, absorbed by LN2
SC_MIX8 = 4.0                # fp8 storage scale for mixed
SC_W = 64.0                  # fp8 weight scale
SC_OT = 32.0                 # ocat fp8 range scale (via ones = 1/32)
FP16 = np.float16
FP8 = ml_dtypes.float8_e4m3

_COMPILED = None


def _build():
    import concourse.bass as bass  # noqa: F401
    import concourse.tile as tile
    from concourse import bacc, mybir

    f16 = mybir.dt.float16
    f8 = mybir.dt.float8e4
    f32 = mybir.dt.float32
    Alu = mybir.AluOpType
    Act = mybir.ActivationFunctionType
    DR = mybir.MatmulPerfMode.DoubleRow

    nc = bacc.Bacc("TRN2", target_bir_lowering=False, debug=False,
                   enable_asserts=True, num_devices=N_CORES)

    xt = nc.dram_tensor("xt", [CPC, H, CHUNK], f16, kind="ExternalInput")
    kernT = nc.dram_tensor("kernt", [H, W], f16, kind="ExternalInput")
    w1a = nc.dram_tensor("w1a", [W + 1, HM], f16, kind="ExternalInput")
    gln = nc.dram_tensor("gln", [128, KM], f32, kind="ExternalInput")
    bln = nc.dram_tensor("bln", [128, KM], f32, kind="ExternalInput")
    w2 = nc.dram_tensor("w2", [HM, H], f16, kind="ExternalInput")
    b2c = nc.dram_tensor("b2c", [128, KH], f32, kind="ExternalInput")
    b2r = nc.dram_tensor("b2r", [128, KH], f32, kind="ExternalInput")
    wq8 = nc.dram_tensor("wq8", [H, H], f8, kind="ExternalInput")
    wk8 = nc.dram_tensor("wk8", [H, H], f8, kind="ExternalInput")
    wv8 = nc.dram_tensor("wv8", [H, H], f8, kind="ExternalInput")
    wo8 = nc.dram_tensor("wo8", [H, H], f8, kind="ExternalInput")
    identD = nc.dram_tensor("ident", [128, 128], f16, kind="ExternalInput")
    gw = nc.dram_tensor("gw", [H, G], f16, kind="ExternalInput")
    bw = nc.dram_tensor("bw", [128, G], f32, kind="ExternalInput")
    y = nc.dram_tensor("y", [CPC, CHUNK, G], f32, kind="ExternalOutput")

    with tile.TileContext(nc) as tc:
        with (
            tc.tile_pool(name="wp", bufs=1) as wp,
            tc.tile_pool(name="ws", bufs=1) as ws,
            tc.tile_pool(name="sm", bufs=2) as sm,
            tc.tile_pool(name="ps", bufs=3, space="PSUM") as ps,
            tc.tile_pool(name="ps2", bufs=2, space="PSUM") as ps2,
        ):
            # ---------- persistent weights ----------
            kt_sb = wp.tile([128, KH, W], f16, tag="ktw")
            nc.sync.dma_start(kt_sb[:], kernT.ap().rearrange("(i p) w -> p i w", p=128))
            w1a_sb = wp.tile([W + 1, HM], f16, tag="w1a")
            nc.sync.dma_start(w1a_sb[:], w1a.ap())
            gln_sb = wp.tile([128, KM], f32, tag="gln")
            nc.sync.dma_start(gln_sb[:], gln.ap())
            bln_sb = wp.tile([128, KM], f32, tag="bln")
            nc.sync.dma_start(bln_sb[:], bln.ap())
            b2_sb = wp.tile([128, KH], f32, tag="b2")
            nc.sync.dma_start(b2_sb[:], b2c.ap())
            b2r_sb = wp.tile([128, KH], f32, tag="b2r")
            nc.sync.dma_start(b2r_sb[:], b2r.ap())
            w2_sb = wp.tile([128, KM, H], f16, tag="w2s")
            wq_sb = wp.tile([128, KH, H], f8, tag="wq")
            wk_sb = wp.tile([128, KH, H], f8, tag="wk")
            wv_sb = wp.tile([128, KH, H], f8, tag="wv")
            wo_sb = wp.tile([128, KH, H], f8, tag="wo")
            id_sb = wp.tile([128, 128], f16, tag="ident")
            gw_sb = wp.tile([128, KH, G], f16, tag="gw")
            bw_sb = wp.tile([128, G], f32, tag="bw")

            def load_weights():
                # emitted after the input loads so x doesn't queue behind 6MB
                nc.sync.dma_start(w2_sb[:], w2.ap().rearrange("(i p) m -> p i m", p=128))
                nc.sync.dma_start(wq_sb[:], wq8.ap().rearrange("(i p) m -> p i m", p=128))
                nc.sync.dma_start(wk_sb[:], wk8.ap().rearrange("(i p) m -> p i m", p=128))
                nc.sync.dma_start(wv_sb[:], wv8.ap().rearrange("(i p) m -> p i m", p=128))
                nc.sync.dma_start(wo_sb[:], wo8.ap().rearrange("(i p) m -> p i m", p=128))
                nc.sync.dma_start(id_sb[:], identD.ap())
                nc.sync.dma_start(gw_sb[:], gw.ap().rearrange("(i p) g -> p i g", p=128))
                nc.sync.dma_start(bw_sb[:], bw.ap())
            ones2 = wp.tile([128, 2, 1], f8, tag="ones2")
            nc.vector.memset(ones2[:], 1.0 / SC_OT)
            eps_sb = wp.tile([128, 1], f32, tag="eps")
            nc.vector.memset(eps_sb[:], EPS)
            eps2_sb = wp.tile([128, 1], f32, tag="eps2")
            nc.vector.memset(eps2_sb[:], EPS * SC_RES * SC_RES)

            St = [dict() for _ in range(CPC)]

            # ---------- stage 1: input load + wavelet coeffs ----------
            def st1(c):
                xts = ws.tile([128, KH, CHUNK], f16, tag="A", bufs=2)
                for ii in range(KH):
                    nc.sync.dma_start(
                        xts[:, ii:ii + 1, :],
                        xt.ap()[c, ii * 128:(ii + 1) * 128, :].rearrange(
                            "(i p) t -> p i t", p=128))
                coef = ws.tile([W + 1, CHUNK], f16, tag="coef", bufs=2)
                nc.gpsimd.memset(coef[:, :], 1.0)  # row W = folded mix_b1
                for n in range(2):
                    cps = ps.tile([128, 512], f32, tag="mm")
                    for i in range(KH):
                        nc.tensor.matmul(cps[:W, :], kt_sb[:, i, :],
                                         xts[:, i, n * 512:(n + 1) * 512],
                                         start=(i == 0), stop=(i == KH - 1))
                    nc.scalar.copy(coef[:W, n * 512:(n + 1) * 512], cps[:W, :])
                St[c]["coef"] = coef

            # ---------- stage 2: w1 + LN1 + gelu -> hidT ----------
            def st2(c):
                coef = St[c]["coef"]
                hidT = ws.tile([128, KM, CHUNK], f16, tag="hidT", bufs=1)
                for t in range(NT):
                    hps = ps.tile([128, HM], f32, tag="mm")
                    nc.tensor.matmul(hps[:], coef[:, t * 128:(t + 1) * 128],
                                     w1a_sb[:], start=True, stop=True)
                    st6 = sm.tile([128, 6], f32, tag="st6")
                    nc.vector.bn_stats(st6[:], hps[:])
                    mv = sm.tile([128, 2], f32, tag="mv")
                    nc.vector.bn_aggr(mv[:], st6[:])
                    sq = sm.tile([128, 1], f32, tag="sq")
                    nc.scalar.activation(sq[:], mv[:, 1:2], Act.Sqrt, bias=eps_sb[:])
                    iv = sm.tile([128, 1], f32, tag="iv")
                    nc.vector.reciprocal(iv[:], sq[:])
                    tmp = sm.tile([128, HM], f16, tag="mtmp")
                    nc.vector.tensor_scalar(tmp[:], hps[:], mv[:, 0:1], iv[:],
                                            op0=Alu.subtract, op1=Alu.mult)
                    nc.sync.dma_start_transpose(hidT[:, :, t * 128:(t + 1) * 128],
                                                tmp[:])
                # gamma/beta fused into gelu via per-partition scale/bias
                for ki in range(KM):
                    nc.scalar.activation(hidT[:, ki, :], hidT[:, ki, :], Act.Gelu,
                                         bias=bln_sb[:, ki:ki + 1],
                                         scale=gln_sb[:, ki:ki + 1])
                St[c]["hidT"] = hidT

            # ---------- stage 3: w2 GEMM -> mixT16 (chunked by m for overlap) ----------
            def st3_mm(c, m_lo, m_hi):
                hidT = St[c]["hidT"]
                if "mixT16" not in St[c]:
                    St[c]["mixT16"] = ws.tile([128, KH, CHUNK], f16, tag="B", bufs=2, name="mixT16")
                mixT16 = St[c]["mixT16"]
                if "mix8" not in St[c]:
                    St[c]["mix8"] = ws.tile([128, KH, CHUNK], f8, tag="E8",
                                            bufs=2, name="mix8")
                mix8 = St[c]["mix8"]
                for m in range(m_lo, m_hi):
                    for n in range(2):
                        mps = ps.tile([128, 512], f32, tag="mm")
                        for ki in range(KM):
                            nc.tensor.matmul(mps[:], w2_sb[:, ki, m * 128:(m + 1) * 128],
                                             hidT[:, ki, n * 512:(n + 1) * 512],
                                             start=(ki == 0), stop=(ki == KM - 1))
                        if c == 0:
                            nc.scalar.activation(
                                mixT16[:, m, n * 512:(n + 1) * 512], mps[:],
                                Act.Identity, bias=b2r_sb[:, m:m + 1],
                                scale=SC_RES)
                        else:
                            nc.vector.tensor_scalar(
                                mixT16[:, m, n * 512:(n + 1) * 512],
                                mps[:], b2_sb[:, m:m + 1], SC_RES,
                                op0=Alu.add, op1=Alu.mult)
                        nc.vector.tensor_scalar(mix8[:, m, n * 512:(n + 1) * 512],
                                                mps[:], b2_sb[:, m:m + 1], SC_MIX8,
                                                op0=Alu.add, op1=Alu.mult)

            def st3_post(c):
                mixT16 = St[c]["mixT16"]
                mixN = ws.tile([128, NT, H], f16, tag="mixN", bufs=2)
                for m in range(KH):
                    nc.sync.dma_start_transpose(mixN[:, :, m * 128:(m + 1) * 128],
                                                mixT16[:, m, :])
                St[c]["mixN"] = mixN

            # ---------- stage 4: q/k/v projections (fp8 double-row) ----------
            def st4(c):
                mix8 = St[c]["mix8"]
                qT8 = ws.tile([128, KH, CHUNK], f8, tag="Q8", bufs=1)
                kT8 = ws.tile([128, KH, CHUNK], f8, tag="K8", bufs=1)
                for (dst, wsb, on_act) in ((qT8, wq_sb, True), (kT8, wk_sb, False)):
                    for m in range(KH):
                        qps = ps2.tile([128, CHUNK], f32, tag="wide")
                        for n in range(2):
                            for kj in range(KP):
                                nc.tensor.matmul(
                                    qps[:, n * 512:(n + 1) * 512],
                                    wsb[:, 2 * kj:2 * kj + 2, m * 128:(m + 1) * 128],
                                    mix8[:, 2 * kj:2 * kj + 2, n * 512:(n + 1) * 512],
                                    start=(kj == 0), stop=(kj == KP - 1),
                                    perf_mode=DR)
                        if on_act:
                            nc.scalar.mul(dst[:, m, :], qps[:], 1.0 / SC_W)
                        else:
                            nc.vector.tensor_scalar(dst[:, m, :], qps[:],
                                                    1.0 / SC_W, None, op0=Alu.mult)
                vN8 = ws.tile([128, NT, H], f8, tag="V8", bufs=1)
                for t in range(NT):
                    vps = ps2.tile([128, CHUNK], f32, tag="wide")
                    for n in range(2):
                        for kj in range(KP):
                            nc.tensor.matmul(
                                vps[:, n * 512:(n + 1) * 512],
                                mix8[:, 2 * kj:2 * kj + 2, t * 128:(t + 1) * 128],
                                wv_sb[:, 2 * kj:2 * kj + 2, n * 512:(n + 1) * 512],
                                start=(kj == 0), stop=(kj == KP - 1),
                                perf_mode=DR)
                    nc.scalar.mul(vN8[:, t, :], vps[:], 1.0 / SC_W)
                St[c]["qT8"], St[c]["kT8"], St[c]["vN8"] = qT8, kT8, vN8

            # ---------- stage 5: attention ----------
            def sc_exp(c, h):
                qT8, kT8 = St[c]["qT8"], St[c]["kT8"]
                et8 = ws.tile([128, KH, CHUNK], f8, tag="E8", bufs=2)
                for kt in range(NT):
                    stp = ps2.tile([128, CHUNK], f32, tag="wide")
                    for qn in range(2):
                        nc.tensor.matmul(
                            stp[:, qn * 512:(qn + 1) * 512],
                            kT8[:, 2 * h:2 * h + 2, kt * 128:(kt + 1) * 128],
                            qT8[:, 2 * h:2 * h + 2, qn * 512:(qn + 1) * 512],
                            start=True, stop=True, perf_mode=DR)
                    # psum holds (4q.4k)=16*qk
                    nc.scalar.activation(et8[:, kt, :], stp[:], Act.Exp,
                                         scale=float(HD ** -0.5 / 16.0))
                St[c]["et8"] = et8

            def pv(c, h):
                et8, vN8 = St[c]["et8"], St[c]["vN8"]
                if "ocat" not in St[c]:
                    St[c]["ocat"] = ws.tile([128, NT, H], f16, tag="A", bufs=2, name="ocat")
                ocat = St[c]["ocat"]
                for qt in range(NT):
                    ovp = ps.tile([128, 512], f32, tag="mm")
                    for kj in range(KP):
                        nc.tensor.matmul(ovp[:, :HD],
                                         et8[:, 2 * kj:2 * kj + 2,
                                             qt * 128:(qt + 1) * 128],
                                         vN8[:, 2 * kj:2 * kj + 2,
                                             h * HD:(h + 1) * HD],
                                         start=(kj == 0), stop=(kj == KP - 1),
                                         perf_mode=DR)
                        # denominator column rides in the same bank; kj==0
                        # start=True above cleared it, so keep start=False.
                        nc.tensor.matmul(ovp[:, HD:HD + 1],
                                         et8[:, 2 * kj:2 * kj + 2,
                                             qt * 128:(qt + 1) * 128],
                                         ones2[:],
                                         start=False, stop=(kj == KP - 1),
                                         perf_mode=DR, skip_group_check=True)
                    rq = sm.tile([128, 1], f32, tag="rq")
                    nc.vector.reciprocal(rq[:], ovp[:, HD:HD + 1])
                    # ocat = SC_OT*SC_MIX8*o (ones=1/32 baked the 32x)
                    nc.vector.tensor_scalar(ocat[:, qt, h * HD:(h + 1) * HD],
                                            ovp[:, :HD], rq[:], None, op0=Alu.mult)

            # ---------- stage 6 pre: transpose ocat + fp8 cast ----------
            def st6_pre(c):
                ocat = St[c]["ocat"]
                otc = ws.tile([128, KH, CHUNK], f16, tag="A", bufs=2)
                zT = ws.tile([128, KH, CHUNK], f16, tag="A", bufs=2)
                for qt in range(NT):
                    nc.sync.dma_start_transpose(otc[:, :, qt * 128:(qt + 1) * 128],
                                                ocat[:, qt, :])
                otc8 = ws.tile([128, KH, CHUNK], f8, tag="O8y", bufs=1)
                for m in range(4):
                    if c == 0:
                        nc.vector.tensor_copy(otc8[:, m, :], otc[:, m, :])
                    else:
                        nc.scalar.copy(otc8[:, m, :], otc[:, m, :])
                for mh in range(2):
                    nc.gpsimd.tensor_copy(otc8[:, 4 + 2 * mh:6 + 2 * mh, :],
                                          otc[:, 4 + 2 * mh:6 + 2 * mh, :])
                St[c]["otc8"], St[c]["zT"] = otc8, zT

            # ---------- stage 6: wo + residual(PE) + LN2 + z, per token tile ----------
            def wo_ln2(c, t_lo, t_hi):
                otc8, mixN, zT = St[c]["otc8"], St[c]["mixN"], St[c]["zT"]
                if "z" not in St[c]:
                    St[c]["z"] = ws.tile([128, NT, H], f16, tag="B", bufs=2, name="z")
                z = St[c]["z"]
                for t in range(t_lo, t_hi):
                    ops2 = ps2.tile([128, CHUNK], f32, tag="wide")
                    for n in range(2):
                        for fi in range(KP):
                            nc.tensor.matmul(
                                ops2[:, n * 512:(n + 1) * 512],
                                otc8[:, 2 * fi:2 * fi + 2, t * 128:(t + 1) * 128],
                                wo_sb[:, 2 * fi:2 * fi + 2, n * 512:(n + 1) * 512],
                                start=(fi == 0), stop=False, perf_mode=DR)
                        # residual: += I.T @ mixN on the PE (both SC_RES-scaled)
                        nc.tensor.matmul(ops2[:, n * 512:(n + 1) * 512], id_sb[:],
                                         mixN[:, t, n * 512:(n + 1) * 512],
                                         start=False, stop=True)
                    st6b = sm.tile([128, 2, 6], f32, tag="st6b")
                    for half in range(2):
                        nc.vector.bn_stats(st6b[:, half, :],
                                           ops2[:, half * 512:(half + 1) * 512])
                    mv2 = sm.tile([128, 2], f32, tag="mv")
                    nc.vector.bn_aggr(mv2[:], st6b[:])
                    sq2 = sm.tile([128, 1], f32, tag="sq")
                    nc.scalar.activation(sq2[:], mv2[:, 1:2], Act.Sqrt,
                                         bias=eps2_sb[:])
                    iv2 = sm.tile([128, 1], f32, tag="iv")
                    nc.vector.reciprocal(iv2[:], sq2[:])
                    if c == 0:
                        nc.vector.tensor_scalar(z[:, t, :], ops2[:], mv2[:, 0:1],
                                                iv2[:], op0=Alu.subtract,
                                                op1=Alu.mult)
                    else:
                        nmi = sm.tile([128, 1], f32, tag="nmi")
                        nc.vector.tensor_scalar(nmi[:], mv2[:, 0:1], iv2[:], -1.0,
                                                op0=Alu.mult, op1=Alu.mult)
                        nc.scalar.activation(z[:, t, :], ops2[:], Act.Identity,
                                             bias=nmi[:], scale=iv2[:])
                    nc.sync.dma_start_transpose(zT[:, :, t * 128:(t + 1) * 128],
                                                z[:, t, :])

            # ---------- stage 7: output projection (fp16) ----------
            def st7(c, t_lo, t_hi):
                zT = St[c]["zT"]
                if "ych" not in St[c]:
                    St[c]["ych"] = ws.tile([128, NT, G], f32, tag="O8y", bufs=1, name="ych")
                ych = St[c]["ych"]
                for t in range(t_lo, t_hi):
                    yps = ps.tile([128, 512], f32, tag="mm")
                    for fi in range(KH):
                        nc.tensor.matmul(yps[:, :G], zT[:, fi, t * 128:(t + 1) * 128],
                                         gw_sb[:, fi, :],
                                         start=(fi == 0), stop=(fi == KH - 1))
                    nc.vector.tensor_add(ych[:, t, :], yps[:, :G], bw_sb[:])

            def yout(c):
                ych = St[c]["ych"]
                for hh in range(2):
                    nc.sync.dma_start(
                        y.ap()[c, hh * 512:(hh + 1) * 512, :].rearrange(
                            "(t p) g -> p t g", p=128),
                        ych[:, hh * 4:(hh + 1) * 4, :])

            # ================= emission schedule (software pipeline) =================
            st1(0)
            st1(1)
            load_weights()
            st2(0)
            st3_mm(0, 0, KH)
            st3_post(0)
            st4(0)
            st2(1)
            # chunk0 attention; chunk1 mixer GEMMs fill the exp-bound PE gaps
            sc_exp(0, 0); st3_mm(1, 0, 4); pv(0, 0)
            sc_exp(0, 1); st3_mm(1, 4, KH); pv(0, 1)
            sc_exp(0, 2); st3_post(1); pv(0, 2)
            sc_exp(0, 3); pv(0, 3)
            st4(1)
            st6_pre(0)
            # chunk0 output stages; chunk1 attention fills the gaps
            sc_exp(1, 0); wo_ln2(0, 0, 4); pv(1, 0)
            sc_exp(1, 1); wo_ln2(0, 4, NT); pv(1, 1)
            sc_exp(1, 2); st7(0, 0, 4); pv(1, 2)
            sc_exp(1, 3); st7(0, 4, NT); yout(0); pv(1, 3)
            st6_pre(1)
            wo_ln2(1, 0, NT)
            st7(1, 0, NT)
            yout(1)

    nc.compile()
    return nc


def _get_compiled():
    global _COMPILED
    if _COMPILED is None:
        _COMPILED = _build()
    return _COMPILED


def _prep_inputs(inputs):
    f32 = np.float32

    def a(name):
        return np.asarray(inputs[name], dtype=f32)

    x = a("x")
    mw = a("mother_wavelets")
    scales = a("scales")
    norm = np.sqrt(np.sum(mw ** 2, axis=2, keepdims=True))
    kern = (mw / np.maximum(norm, 1e-12)) * (1.0 / (1.0 + np.exp(-scales)))
    kern = kern[0, :, :, 0]                      # (W, H)
    kernT = np.ascontiguousarray(kern.T).astype(FP16)

    w1a = np.concatenate([a("mix_w1"), a("mix_b1")[None, :]], axis=0).astype(FP16)
    gln = np.ascontiguousarray(a("mix_ln_g").reshape(KM, 128).T).astype(f32)
    bln = np.ascontiguousarray(a("mix_ln_b").reshape(KM, 128).T).astype(f32)
    w2 = a("mix_w2").astype(FP16)
    b2c = np.ascontiguousarray(a("mix_b2").reshape(KH, 128).T).astype(f32)
    b2r = (b2c * SC_RES).astype(f32)
    gw = (a("out_ln_g")[:, None] * a("out_w")).astype(FP16)
    bw_vec = a("out_ln_b") @ a("out_w") + a("out_b")
    bw = np.tile(bw_vec[None, :], (128, 1)).astype(f32)

    shared = {
        "kernt": kernT, "w1a": w1a, "gln": gln, "bln": bln, "w2": w2,
        "b2c": b2c, "b2r": b2r,
        "wq8": (a("wq") * SC_W).astype(FP8), "wk8": (a("wk") * SC_W).astype(FP8),
        "wv8": (a("wv") * SC_W).astype(FP8), "wo8": (a("wo") * SC_W).astype(FP8),
        "ident": np.eye(128, dtype=FP16),
        "gw": gw, "bw": bw,
    }

    xc = x.reshape(N_CHUNKS, CHUNK, H)
    xt_all = np.ascontiguousarray(xc.transpose(0, 2, 1)).astype(FP16)  # (16, H, CHUNK)
    in_maps = []
    for core in range(N_CORES):
        m = dict(shared)
        m["xt"] = np.ascontiguousarray(xt_all[core * CPC:(core + 1) * CPC])
        in_maps.append(m)
    return in_maps


def kernel(**inputs) -> np.ndarray:
    from concourse.bass_utils import run_bass_kernel_spmd

    nc = _get_compiled()
    in_maps = _prep_inputs(inputs)
    res = run_bass_kernel_spmd(nc, in_maps, core_ids=list(range(N_CORES)))
    out = np.concatenate([r["y"] for r in res.results], axis=0)  # (16, CHUNK, G)
    return out.reshape(B, S, G).astype(np.float32)
